# revision 32
# baseline (speedup 1.0000x reference)
"""Trainium2 Bass kernel for bilinear causal attention (no softmax).

Math (from the reference):
  Omega[b,h,t,u] = r_t^T Q^h r_u            (scores)
  out[b,t,:]     = sum_h sum_{u<=t} Omega[b,h,t,u] * (E^h r_u)

Because there is no softmax, the causal sum factorizes exactly
(linear-attention chunking).  With V_h[u,:] = E_h r_u, k_t = Q_h^T r_t,
and chunks of 128 along t:

  out[t] = sum_h  k_t^T M_h^{(<c)}  +  sum_{u in chunk c, u<=t} (k_t.r_u) V_h[u]
  M_h^{(<c)} = sum_{u < 128c} r_u V_h[u]^T          (256x256 running state)

This is ~2.1x fewer PE cycles than materializing all causal Omega tiles.

Shapes: r_prime [1,4,2048,256] f32, Q [1,8,256,256], E [1,8,256,256],
out [1,4,2048,256] f32.

Sharding over 8 NeuronCores: core = 2*b + hg handles batch b (4 batches)
and head-group hg (heads hg*4 .. hg*4+3).  Each core produces a partial
output summed over its 4 heads; the host adds the two head-group
partials per batch.  No on-chip collectives.

Per-core dataflow (all matmuls bf16 with f32 PSUM accumulation):
  phase A:  KT_h[i',t] = sum_i Q[i,i'] rT[i,t]      ([256,2048] per head)
            V[u,(sh,i)] = sum_j rT[j,u] ET[j,(sh,i)] (head-pair packed)
  phase B per 128-chunk c:
            ST[u,(h,t)] = sum_{i'} rT[i',u] KT4[i',(h,t)]  (4 heads packed)
            mask (triu in [u,t]) -> st_sb bf16
            ot[t,i]  = sum_h ST_h^T V_h  +  sum_h KT_h^T M_h   (PSUM accum)
            M_h     += r_c^T V_h     (persistent PSUM banks, start=c==0)
            copy M -> SBUF bf16 for the next chunk's cross term
Output is produced t-major ([t,i]) so no host transpose is needed.
"""

import numpy as np
import ml_dtypes

N_T = 2048           # sequence length
N_IN = 256           # feature dim (i, j, i' all 256)
CH = 128             # chunk size along t/u
NCH = N_T // CH      # 16 chunks
N_CORES = 8

_CACHE = {}


def _build_nc():
    from concourse import mybir, bacc, tile

    BF16 = mybir.dt.bfloat16
    F32 = mybir.dt.float32

    nc = bacc.Bacc(
        "TRN2", target_bir_lowering=False, debug=False, num_devices=N_CORES
    )
    # Inputs partition-major with long contiguous per-partition runs so the
    # DGE descriptors are 1-8KB (descriptor count, not bytes, dominates the
    # cost of small-run DMAs).
    # rt: [p=i%128, t-512-chunk, i//128, t%512]
    rt_d = nc.dram_tensor("rt", [128, 4, 2, 512], BF16, kind="ExternalInput").ap()
    ru_d = nc.dram_tensor("ru", [128, NCH, N_IN], BF16, kind="ExternalInput").ap()
    q_d = nc.dram_tensor("q", [128, 4, 2, N_IN], BF16, kind="ExternalInput").ap()
    # et pairs two heads side by side: [p=j, head-pair, j-chunk, (sh,i)=512]
    et_d = nc.dram_tensor("et", [128, 2, 2, 512], BF16, kind="ExternalInput").ap()
    mask_d = nc.dram_tensor("cmask", [128, 512], F32, kind="ExternalInput").ap()
    # t-major output: [t-chunk, t-in-chunk, i]; bf16 partials, summed on host
    out_d = nc.dram_tensor("out", [NCH, 128, N_IN], BF16, kind="ExternalOutput").ap()

    with tile.TileContext(nc) as tc:
        with (
            tc.tile_pool(name="consts", bufs=1) as consts,
            tc.tile_pool(name="stsb", bufs=8) as stp,
            tc.tile_pool(name="outsb", bufs=3) as outp,
            tc.tile_pool(name="psum", bufs=1, space="PSUM") as psum,
        ):
            rt_sb = consts.tile([128, 4, 2, 512], BF16)
            ru_sb = consts.tile([128, NCH, N_IN], BF16)
            q_sb = consts.tile([128, 4, 2, N_IN], BF16)
            et_sb = consts.tile([128, 2, 2, 512], BF16)
            mask_sb = consts.tile([128, 512], F32)
            # KT: [p=i', i'-tile, t-chunk, h, t-in-chunk]
            kt_sb = consts.tile([128, 2, NCH, 4, CH], BF16)
            # V: [p=u, u-chunk, head-pair, (sh,i)]
            v_sb = consts.tile([128, NCH, 2, 512], BF16)
            # M state snapshot (bf16), double-buffered by chunk parity so
            # chunk c's snapshot write never waits on chunk c's cross-term
            # read of the previous snapshot:
            # [p=i', buf, head-pair, i'-tile, (sh,i)]
            m_sb = consts.tile([128, 2, 2, 2, 512], BF16)

            # PE warm-up junk matmuls need a zeroed SBUF tile; vector's
            # sequencer clears the startup barrier earliest of the engines
            # that can memset.
            junk_sb = consts.tile([128, 640], BF16)
            nc.vector.memset(junk_sb[:], 0.0)

            # Input DMAs.  gpsimd's SWDGE queue streams at 150-250 GB/s while
            # the sync/scalar HWDGE queues manage ~40-65 GB/s, so everything
            # phase A needs goes through gpsimd in need-order; sync/scalar
            # only carry small q heads + the mask.
            # q rides gpsimd in fine-grained pieces ordered exactly by KT's
            # consumption order, so the first KT group only waits ~64KB.
            nc.gpsimd.dma_start(out=q_sb[:, 0, 0], in_=q_d[:, 0, 0])
            nc.sync.dma_start(out=rt_sb[:, 0, 0, :], in_=rt_d[:, 0, 0, :])
            nc.scalar.dma_start(out=rt_sb[:, 0, 1, :], in_=rt_d[:, 0, 1, :])
            nc.gpsimd.dma_start(out=q_sb[:, 0, 1], in_=q_d[:, 0, 1])
            nc.gpsimd.dma_start(out=q_sb[:, 1, 0], in_=q_d[:, 1, 0])
            nc.gpsimd.dma_start(out=q_sb[:, 1, 1], in_=q_d[:, 1, 1])
            nc.gpsimd.dma_start(out=q_sb[:, 2], in_=q_d[:, 2])
            nc.gpsimd.dma_start(out=q_sb[:, 3], in_=q_d[:, 3])
            nc.gpsimd.dma_start(out=rt_sb[:, 1], in_=rt_d[:, 1])
            nc.gpsimd.dma_start(out=rt_sb[:, 2:4], in_=rt_d[:, 2:4])
            nc.gpsimd.dma_start(out=et_sb[:], in_=et_d[:])
            nc.gpsimd.dma_start(out=ru_sb[:], in_=ru_d[:])
            nc.sync.dma_start(out=mask_sb[:], in_=mask_d[:])

            # Junk matmuls fill the DMA wait and lift the HAM clock gate.
            for _ in range(8):
                junk_ps = psum.tile([128, 512], F32, tag="ps512", bufs=2)
                nc.tensor.matmul(
                    junk_ps[:], junk_sb[:, 0:128], junk_sb[:, 128:640],
                    start=True, stop=True,
                )

            # Phase A: KT per head, V per head-pair, 512-wide moving chunks.
            cp_i = [0]

            def _any_copy(dst, src):
                cp_i[0] += 1
                if cp_i[0] % 2 == 0:
                    nc.vector.tensor_copy(dst, src)
                else:
                    nc.scalar.copy(dst, src)

            # KT first (needs only q + rt, the earliest DMAs), rotating
            # through the 4 "m" banks (idle until phase B); then V (needs
            # et) on the 2 "ps512" banks.  The 4-deep/2-deep rotations give
            # the psum eviction copies enough slack to never stall the PE.
            for tcn in range(4):
                for h in range(4):
                    for ipc in range(2):
                        kt_ps = psum.tile(
                            [128, 512], F32, tag="m", bufs=4, name="kt_ps"
                        )
                        for ic in range(2):
                            nc.tensor.matmul(
                                kt_ps[:],
                                q_sb[:, h, ic, ipc * 128 : (ipc + 1) * 128],
                                rt_sb[:, tcn, ic, :],
                                start=(ic == 0),
                                stop=(ic == 1),
                            )
                        _any_copy(
                            kt_sb[:, ipc, 4 * tcn : 4 * tcn + 4, h, :],
                            kt_ps[:].rearrange("p (tc tt) -> p tc tt", tc=4),
                        )
            for hp in range(2):
                for ut in range(NCH):
                    v_ps = psum.tile(
                        [128, 512], F32, tag="m", bufs=4, name="v_ps"
                    )
                    for jc in range(2):
                        nc.tensor.matmul(
                            v_ps[:],
                            rt_sb[:, ut // 4, jc, (ut % 4) * 128 : (ut % 4 + 1) * 128],
                            et_sb[:, hp, jc, :],
                            start=(jc == 0),
                            stop=(jc == 1),
                        )
                    _any_copy(v_sb[:, ut, hp, :], v_ps[:])

            # Persistent PSUM accumulators for M (one bank per (hp, i'-tile)).
            m_ps = [
                [
                    psum.tile([128, 512], F32, tag="m", bufs=4, name=f"m_{hp}_{j}")
                    for j in range(2)
                ]
                for hp in range(2)
            ]

            def _st(c):
                st_ps = psum.tile([128, 512], F32, tag="ps512", bufs=2)
                for j in range(2):
                    nc.tensor.matmul(
                        st_ps[:],
                        rt_sb[:, c // 4, j, (c % 4) * 128 : (c % 4 + 1) * 128],
                        kt_sb[:, j, c, :, :].rearrange("p tc tt -> p (tc tt)"),
                        start=(j == 0),
                        stop=(j == 1),
                    )
                st_sb = stp.tile([128, 512], BF16, tag="st")
                nc.vector.tensor_mul(st_sb[:], st_ps[:], mask_sb[:])
                return st_sb

            # Phase B: per 128-chunk, M update first (so its PSUM->SBUF
            # snapshot has a full chunk of PE work to hide behind before the
            # next chunk's cross term reads it), then diagonal block + cross.
            st_cur = _st(0)
            for c in range(NCH):
                if c < NCH - 1:
                    # M update with chunk c, snapshot to SBUF for c+1.
                    for hp in range(2):
                        for j in range(2):
                            nc.tensor.matmul(
                                m_ps[hp][j][:],
                                ru_sb[:, c, j * 128 : (j + 1) * 128],
                                v_sb[:, c, hp, :],
                                start=(c == 0),
                                stop=(c == NCH - 2),
                            )
                    mb = c % 2
                    nc.vector.tensor_copy(m_sb[:, mb, 0, 0, :], m_ps[0][0][:])
                    nc.scalar.copy(m_sb[:, mb, 0, 1, :], m_ps[0][1][:])
                    nc.vector.tensor_copy(m_sb[:, mb, 1, 0, :], m_ps[1][0][:])
                    nc.scalar.copy(m_sb[:, mb, 1, 1, :], m_ps[1][1][:])
                ot = psum.tile([128, N_IN], F32, tag="ot", bufs=2, name=f"ot{c}")
                for h in range(4):
                    nc.tensor.matmul(
                        ot[:],
                        st_cur[:, h * CH : (h + 1) * CH],
                        v_sb[:, c, h // 2, (h % 2) * N_IN : (h % 2 + 1) * N_IN],
                        start=(h == 0),
                        stop=(c == 0 and h == 3),
                    )
                if c < NCH - 1:
                    st_cur = _st(c + 1)
                if c > 0:
                    for h in range(4):
                        for j in range(2):
                            nc.tensor.matmul(
                                ot[:],
                                kt_sb[:, j, c, h, :],
                                m_sb[
                                    :, (c - 1) % 2, h // 2, j,
                                    (h % 2) * N_IN : (h % 2 + 1) * N_IN,
                                ],
                                start=False,
                                stop=(h == 3 and j == 1),
                            )
                o_sb = outp.tile([128, N_IN], BF16, tag="osb")
                nc.scalar.copy(o_sb[:], ot[:])
                # gpsimd carries no out-DMA after chunk 11 so its expensive
                # SWDGE end-of-kernel drain overlaps the remaining compute.
                if c == NCH - 1:
                    nc.sync.dma_start(out=out_d[c, 0:64], in_=o_sb[0:64])
                    nc.scalar.dma_start(out=out_d[c, 64:128], in_=o_sb[64:128])
                elif c >= 12:
                    nc.sync.dma_start(out=out_d[c], in_=o_sb[:])
                else:
                    out_eng = nc.gpsimd if c % 2 == 0 else nc.sync
                    out_eng.dma_start(out=out_d[c], in_=o_sb[:])

    nc.compile()
    return nc


def _get_nc():
    if "nc" not in _CACHE:
        _CACHE["nc"] = _build_nc()
    return _CACHE["nc"]


def _make_cmask():
    # ST layout is [p=u, (h, t-in-chunk)]; keep u <= t -> triu, tiled 4x.
    tri = np.triu(np.ones((128, 128), np.float32))
    return np.ascontiguousarray(np.tile(tri, (1, 4)))


def _make_in_maps(r_prime, Q, E):
    bf16 = ml_dtypes.bfloat16
    cmask = _make_cmask()
    in_maps = []
    for core in range(N_CORES):
        b, hg = core // 2, core % 2
        r = r_prime[0, b]  # [2048, 256]
        # rt[p, tcn, ic, tt] = r[tcn*512+tt, ic*128+p]
        rt = np.ascontiguousarray(
            r.T.reshape(2, 128, 4, 512).transpose(1, 2, 0, 3)
        ).astype(bf16)
        # ru[p, uc, j] = r[uc*128+p, j]
        ru = np.ascontiguousarray(
            r.reshape(NCH, 128, N_IN).transpose(1, 0, 2)
        ).astype(bf16)
        # q[p, h, ic, f] = Q[h, ic*128+p, f]
        qh = np.ascontiguousarray(
            Q[0, hg * 4 : hg * 4 + 4]
            .reshape(4, 2, 128, N_IN)
            .transpose(2, 0, 1, 3)
        ).astype(bf16)
        # et[p, hp, jc, sh*256+f] = E[2hp+sh].T[jc*128+p, f]
        eth = (
            E[0, hg * 4 : hg * 4 + 4]
            .transpose(0, 2, 1)  # [h, j, i']
            .reshape(2, 2, 2, 128, N_IN)  # [hp, sh, jc, p, f]
            .transpose(3, 0, 2, 1, 4)  # [p, hp, jc, sh, f]
            .reshape(128, 2, 2, 512)
        )
        eth = np.ascontiguousarray(eth).astype(bf16)
        in_maps.append({"rt": rt, "ru": ru, "q": qh, "et": eth, "cmask": cmask})
    return in_maps


def _ensure_ntff_hook():
    """The container's `antenv` stub lacks `axon_hooks`, so the boot-time
    NTFF profile hook registration silently no-ops. Recreate it so
    trace=True yields exec_time_ns. Only used by the test harness."""
    import sys
    import types

    if "antenv.axon_hooks" not in sys.modules:
        import antenv

        mod = types.ModuleType("antenv.axon_hooks")
        state = {}
        mod.set_axon_ntff_profile_hook = lambda h: state.update(h=h)
        mod.get_axon_ntff_profile_hook = lambda: state.get("h")
        sys.modules["antenv.axon_hooks"] = mod
        antenv.axon_hooks = mod
    from antenv.axon_hooks import (
        get_axon_ntff_profile_hook,
        set_axon_ntff_profile_hook,
    )

    if get_axon_ntff_profile_hook() is None:
        from trn_agent_boot.trn_boot import _ntff_profile_via_ctypes

        set_axon_ntff_profile_hook(
            _ntff_profile_via_ctypes("/opt/axon/libaxon_pjrt.so")
        )


def _run(r_prime, Q, E, trace=False, trace_kwargs=None):
    from concourse.bass_utils import run_bass_kernel_spmd

    try:
        _ensure_ntff_hook()
    except Exception:
        pass  # profiling is optional; never block the actual run
    r_prime = np.asarray(r_prime, dtype=np.float32)
    Q = np.asarray(Q, dtype=np.float32)
    E = np.asarray(E, dtype=np.float32)
    in_maps = _make_in_maps(r_prime, Q, E)
    nc = _get_nc()
    kw = {}
    if trace:
        kw["trace"] = True
        if trace_kwargs:
            kw.update(trace_kwargs)
    res = run_bass_kernel_spmd(nc, in_maps, core_ids=list(range(N_CORES)), **kw)
    out = np.zeros((1, 4, N_T, N_IN), np.float32)
    for b in range(4):
        p0 = np.asarray(res.results[2 * b]["out"]).astype(np.float32)
        p1 = np.asarray(res.results[2 * b + 1]["out"]).astype(np.float32)
        out[0, b] = (p0 + p1).reshape(N_T, N_IN)
    return out, res


def kernel(r_prime, Q, E):
    out, _ = _run(r_prime, Q, E, trace=False)
    return out


# revision 34
# speedup vs baseline: 1.0221x; 1.0221x over previous
"""Trainium2 Bass kernel for bilinear causal attention (no softmax).

Math (from the reference):
  Omega[b,h,t,u] = r_t^T Q^h r_u            (scores)
  out[b,t,:]     = sum_h sum_{u<=t} Omega[b,h,t,u] * (E^h r_u)

Because there is no softmax, the causal sum factorizes exactly
(linear-attention chunking).  With V_h[u,:] = E_h r_u, k_t = Q_h^T r_t,
and chunks of 128 along t:

  out[t] = sum_h  k_t^T M_h^{(<c)}  +  sum_{u in chunk c, u<=t} (k_t.r_u) V_h[u]
  M_h^{(<c)} = sum_{u < 128c} r_u V_h[u]^T          (256x256 running state)

This is ~2.1x fewer PE cycles than materializing all causal Omega tiles.

Shapes: r_prime [1,4,2048,256] f32, Q [1,8,256,256], E [1,8,256,256],
out [1,4,2048,256] f32.

Sharding over 8 NeuronCores: core = 2*b + hg handles batch b (4 batches)
and head-group hg (heads hg*4 .. hg*4+3).  Each core produces a partial
output summed over its 4 heads; the host adds the two head-group
partials per batch.  No on-chip collectives.

Per-core dataflow (all matmuls bf16 with f32 PSUM accumulation):
  phase A:  KT_h[i',t] = sum_i Q[i,i'] rT[i,t]      ([256,2048] per head)
            V[u,(sh,i)] = sum_j rT[j,u] ET[j,(sh,i)] (head-pair packed)
  phase B per 128-chunk c:
            ST[u,(h,t)] = sum_{i'} rT[i',u] KT4[i',(h,t)]  (4 heads packed)
            mask (triu in [u,t]) -> st_sb bf16
            ot[t,i]  = sum_h ST_h^T V_h  +  sum_h KT_h^T M_h   (PSUM accum)
            M_h     += r_c^T V_h     (persistent PSUM banks, start=c==0)
            copy M -> SBUF bf16 for the next chunk's cross term
Output is produced t-major ([t,i]) so no host transpose is needed.
"""

import numpy as np
import ml_dtypes

N_T = 2048           # sequence length
N_IN = 256           # feature dim (i, j, i' all 256)
CH = 128             # chunk size along t/u
NCH = N_T // CH      # 16 chunks
N_CORES = 8

_CACHE = {}


def _build_nc():
    from concourse import mybir, bacc, tile

    BF16 = mybir.dt.bfloat16
    F32 = mybir.dt.float32

    nc = bacc.Bacc(
        "TRN2", target_bir_lowering=False, debug=False, num_devices=N_CORES
    )
    # Inputs partition-major with long contiguous per-partition runs so the
    # DGE descriptors are 1-8KB (descriptor count, not bytes, dominates the
    # cost of small-run DMAs).
    # rt: [p=i%128, t-512-chunk, i//128, t%512]
    rt_d = nc.dram_tensor("rt", [128, 4, 2, 512], BF16, kind="ExternalInput").ap()
    ru_d = nc.dram_tensor("ru", [128, NCH, N_IN], BF16, kind="ExternalInput").ap()
    q_d = nc.dram_tensor("q", [128, 4, 2, N_IN], BF16, kind="ExternalInput").ap()
    # et pairs two heads side by side: [p=j, head-pair, j-chunk, (sh,i)=512]
    et_d = nc.dram_tensor("et", [128, 2, 2, 512], BF16, kind="ExternalInput").ap()
    mask_d = nc.dram_tensor("cmask", [128, 512], F32, kind="ExternalInput").ap()
    # t-major output: [t-chunk, t-in-chunk, i]; bf16 partials, summed on host
    out_d = nc.dram_tensor("out", [NCH, 128, N_IN], BF16, kind="ExternalOutput").ap()

    with tile.TileContext(nc) as tc:
        with (
            tc.tile_pool(name="consts", bufs=1) as consts,
            tc.tile_pool(name="stsb", bufs=8) as stp,
            tc.tile_pool(name="outsb", bufs=3) as outp,
            tc.tile_pool(name="psum", bufs=1, space="PSUM") as psum,
        ):
            rt_sb = consts.tile([128, 4, 2, 512], BF16)
            ru_sb = consts.tile([128, NCH, N_IN], BF16)
            q_sb = consts.tile([128, 4, 2, N_IN], BF16)
            et_sb = consts.tile([128, 2, 2, 512], BF16)
            mask_sb = consts.tile([128, 512], F32)
            # KT: [p=i', i'-tile, t-chunk, h, t-in-chunk]
            kt_sb = consts.tile([128, 2, NCH, 4, CH], BF16)
            # V: [p=u, u-chunk, head-pair, (sh,i)]
            v_sb = consts.tile([128, NCH, 2, 512], BF16)
            # M state snapshot (bf16), double-buffered by chunk parity so
            # chunk c's snapshot write never waits on chunk c's cross-term
            # read of the previous snapshot:
            # [p=i', buf, head-pair, i'-tile, (sh,i)]
            m_sb = consts.tile([128, 2, 2, 2, 512], BF16)

            # PE warm-up junk matmuls need a zeroed SBUF tile; vector's
            # sequencer clears the startup barrier earliest of the engines
            # that can memset.
            junk_sb = consts.tile([128, 640], BF16)
            nc.vector.memset(junk_sb[:], 0.0)

            # Input DMAs.  gpsimd's SWDGE queue streams at 150-250 GB/s while
            # the sync/scalar HWDGE queues manage ~40-65 GB/s, so everything
            # phase A needs goes through gpsimd in need-order; sync/scalar
            # only carry small q heads + the mask.
            # q rides gpsimd in fine-grained pieces ordered exactly by KT's
            # consumption order, so the first KT group only waits ~64KB.
            nc.gpsimd.dma_start(out=q_sb[:, 0, 0], in_=q_d[:, 0, 0])
            nc.sync.dma_start(out=rt_sb[:, 0, 0, :], in_=rt_d[:, 0, 0, :])
            nc.scalar.dma_start(out=rt_sb[:, 0, 1, :], in_=rt_d[:, 0, 1, :])
            nc.gpsimd.dma_start(out=q_sb[:, 0, 1], in_=q_d[:, 0, 1])
            nc.gpsimd.dma_start(out=q_sb[:, 1, 0], in_=q_d[:, 1, 0])
            nc.gpsimd.dma_start(out=q_sb[:, 1, 1], in_=q_d[:, 1, 1])
            nc.gpsimd.dma_start(out=q_sb[:, 2], in_=q_d[:, 2])
            nc.gpsimd.dma_start(out=q_sb[:, 3], in_=q_d[:, 3])
            nc.gpsimd.dma_start(out=rt_sb[:, 1], in_=rt_d[:, 1])
            nc.gpsimd.dma_start(out=rt_sb[:, 2:4], in_=rt_d[:, 2:4])
            nc.gpsimd.dma_start(out=et_sb[:], in_=et_d[:])
            nc.gpsimd.dma_start(out=ru_sb[:], in_=ru_d[:])
            nc.sync.dma_start(out=mask_sb[:], in_=mask_d[:])

            # Junk matmuls fill the DMA wait and lift the HAM clock gate.
            for _ in range(7):
                junk_ps = psum.tile([128, 512], F32, tag="ps512", bufs=2)
                nc.tensor.matmul(
                    junk_ps[:], junk_sb[:, 0:128], junk_sb[:, 128:640],
                    start=True, stop=True,
                )

            # Phase A: KT per head, V per head-pair, 512-wide moving chunks.
            cp_i = [0]

            def _any_copy(dst, src):
                cp_i[0] += 1
                if cp_i[0] % 2 == 0:
                    nc.vector.tensor_copy(dst, src)
                else:
                    nc.scalar.copy(dst, src)

            # KT first (needs only q + rt, the earliest DMAs), rotating
            # through the 4 "m" banks (idle until phase B); then V (needs
            # et) on the 2 "ps512" banks.  The 4-deep/2-deep rotations give
            # the psum eviction copies enough slack to never stall the PE.
            for tcn in range(4):
                for h in range(4):
                    for ipc in range(2):
                        kt_ps = psum.tile(
                            [128, 512], F32, tag="m", bufs=4, name="kt_ps"
                        )
                        for ic in range(2):
                            nc.tensor.matmul(
                                kt_ps[:],
                                q_sb[:, h, ic, ipc * 128 : (ipc + 1) * 128],
                                rt_sb[:, tcn, ic, :],
                                start=(ic == 0),
                                stop=(ic == 1),
                            )
                        _any_copy(
                            kt_sb[:, ipc, 4 * tcn : 4 * tcn + 4, h, :],
                            kt_ps[:].rearrange("p (tc tt) -> p tc tt", tc=4),
                        )
            for hp in range(2):
                for ut in range(NCH):
                    v_ps = psum.tile(
                        [128, 512], F32, tag="m", bufs=4, name="v_ps"
                    )
                    for jc in range(2):
                        nc.tensor.matmul(
                            v_ps[:],
                            rt_sb[:, ut // 4, jc, (ut % 4) * 128 : (ut % 4 + 1) * 128],
                            et_sb[:, hp, jc, :],
                            start=(jc == 0),
                            stop=(jc == 1),
                        )
                    _any_copy(v_sb[:, ut, hp, :], v_ps[:])

            # Persistent PSUM accumulators for M (one bank per (hp, i'-tile)).
            m_ps = [
                [
                    psum.tile([128, 512], F32, tag="m", bufs=4, name=f"m_{hp}_{j}")
                    for j in range(2)
                ]
                for hp in range(2)
            ]

            def _st(c):
                st_ps = psum.tile([128, 512], F32, tag="ps512", bufs=2)
                for j in range(2):
                    nc.tensor.matmul(
                        st_ps[:],
                        rt_sb[:, c // 4, j, (c % 4) * 128 : (c % 4 + 1) * 128],
                        kt_sb[:, j, c, :, :].rearrange("p tc tt -> p (tc tt)"),
                        start=(j == 0),
                        stop=(j == 1),
                    )
                st_sb = stp.tile([128, 512], BF16, tag="st")
                nc.vector.tensor_mul(st_sb[:], st_ps[:], mask_sb[:])
                return st_sb

            # Phase B: per 128-chunk, M update first (so its PSUM->SBUF
            # snapshot has a full chunk of PE work to hide behind before the
            # next chunk's cross term reads it), then diagonal block + cross.
            st_cur = _st(0)
            for c in range(NCH):
                if c < NCH - 1:
                    # M update with chunk c, snapshot to SBUF for c+1.
                    for hp in range(2):
                        for j in range(2):
                            nc.tensor.matmul(
                                m_ps[hp][j][:],
                                ru_sb[:, c, j * 128 : (j + 1) * 128],
                                v_sb[:, c, hp, :],
                                start=(c == 0),
                                stop=(c == NCH - 2),
                            )
                    mb = c % 2
                    nc.vector.tensor_copy(m_sb[:, mb, 0, 0, :], m_ps[0][0][:])
                    nc.scalar.copy(m_sb[:, mb, 0, 1, :], m_ps[0][1][:])
                    nc.vector.tensor_copy(m_sb[:, mb, 1, 0, :], m_ps[1][0][:])
                    nc.scalar.copy(m_sb[:, mb, 1, 1, :], m_ps[1][1][:])
                ot = psum.tile([128, N_IN], F32, tag="ot", bufs=2, name=f"ot{c}")
                for h in range(4):
                    nc.tensor.matmul(
                        ot[:],
                        st_cur[:, h * CH : (h + 1) * CH],
                        v_sb[:, c, h // 2, (h % 2) * N_IN : (h % 2 + 1) * N_IN],
                        start=(h == 0),
                        stop=(c == 0 and h == 3),
                    )
                if c < NCH - 1:
                    st_cur = _st(c + 1)
                if c > 0:
                    for h in range(4):
                        for j in range(2):
                            nc.tensor.matmul(
                                ot[:],
                                kt_sb[:, j, c, h, :],
                                m_sb[
                                    :, (c - 1) % 2, h // 2, j,
                                    (h % 2) * N_IN : (h % 2 + 1) * N_IN,
                                ],
                                start=False,
                                stop=(h == 3 and j == 1),
                            )
                o_sb = outp.tile([128, N_IN], BF16, tag="osb")
                nc.scalar.copy(o_sb[:], ot[:])
                # gpsimd carries no out-DMA after chunk 11 so its expensive
                # SWDGE end-of-kernel drain overlaps the remaining compute.
                if c == NCH - 1:
                    nc.sync.dma_start(out=out_d[c, 0:64], in_=o_sb[0:64])
                    nc.scalar.dma_start(out=out_d[c, 64:128], in_=o_sb[64:128])
                elif c >= 12:
                    nc.sync.dma_start(out=out_d[c], in_=o_sb[:])
                else:
                    out_eng = nc.gpsimd if c % 2 == 0 else nc.sync
                    out_eng.dma_start(out=out_d[c], in_=o_sb[:])

    nc.compile()
    return nc


def _get_nc():
    if "nc" not in _CACHE:
        _CACHE["nc"] = _build_nc()
    return _CACHE["nc"]


def _make_cmask():
    # ST layout is [p=u, (h, t-in-chunk)]; keep u <= t -> triu, tiled 4x.
    tri = np.triu(np.ones((128, 128), np.float32))
    return np.ascontiguousarray(np.tile(tri, (1, 4)))


def _make_in_maps(r_prime, Q, E):
    bf16 = ml_dtypes.bfloat16
    cmask = _make_cmask()
    in_maps = []
    for core in range(N_CORES):
        b, hg = core // 2, core % 2
        r = r_prime[0, b]  # [2048, 256]
        # rt[p, tcn, ic, tt] = r[tcn*512+tt, ic*128+p]
        rt = np.ascontiguousarray(
            r.T.reshape(2, 128, 4, 512).transpose(1, 2, 0, 3)
        ).astype(bf16)
        # ru[p, uc, j] = r[uc*128+p, j]
        ru = np.ascontiguousarray(
            r.reshape(NCH, 128, N_IN).transpose(1, 0, 2)
        ).astype(bf16)
        # q[p, h, ic, f] = Q[h, ic*128+p, f]
        qh = np.ascontiguousarray(
            Q[0, hg * 4 : hg * 4 + 4]
            .reshape(4, 2, 128, N_IN)
            .transpose(2, 0, 1, 3)
        ).astype(bf16)
        # et[p, hp, jc, sh*256+f] = E[2hp+sh].T[jc*128+p, f]
        eth = (
            E[0, hg * 4 : hg * 4 + 4]
            .transpose(0, 2, 1)  # [h, j, i']
            .reshape(2, 2, 2, 128, N_IN)  # [hp, sh, jc, p, f]
            .transpose(3, 0, 2, 1, 4)  # [p, hp, jc, sh, f]
            .reshape(128, 2, 2, 512)
        )
        eth = np.ascontiguousarray(eth).astype(bf16)
        in_maps.append({"rt": rt, "ru": ru, "q": qh, "et": eth, "cmask": cmask})
    return in_maps


def _ensure_ntff_hook():
    """The container's `antenv` stub lacks `axon_hooks`, so the boot-time
    NTFF profile hook registration silently no-ops. Recreate it so
    trace=True yields exec_time_ns. Only used by the test harness."""
    import sys
    import types

    if "antenv.axon_hooks" not in sys.modules:
        import antenv

        mod = types.ModuleType("antenv.axon_hooks")
        state = {}
        mod.set_axon_ntff_profile_hook = lambda h: state.update(h=h)
        mod.get_axon_ntff_profile_hook = lambda: state.get("h")
        sys.modules["antenv.axon_hooks"] = mod
        antenv.axon_hooks = mod
    from antenv.axon_hooks import (
        get_axon_ntff_profile_hook,
        set_axon_ntff_profile_hook,
    )

    if get_axon_ntff_profile_hook() is None:
        from trn_agent_boot.trn_boot import _ntff_profile_via_ctypes

        set_axon_ntff_profile_hook(
            _ntff_profile_via_ctypes("/opt/axon/libaxon_pjrt.so")
        )


def _run(r_prime, Q, E, trace=False, trace_kwargs=None):
    from concourse.bass_utils import run_bass_kernel_spmd

    try:
        _ensure_ntff_hook()
    except Exception:
        pass  # profiling is optional; never block the actual run
    r_prime = np.asarray(r_prime, dtype=np.float32)
    Q = np.asarray(Q, dtype=np.float32)
    E = np.asarray(E, dtype=np.float32)
    in_maps = _make_in_maps(r_prime, Q, E)
    nc = _get_nc()
    kw = {}
    if trace:
        kw["trace"] = True
        if trace_kwargs:
            kw.update(trace_kwargs)
    res = run_bass_kernel_spmd(nc, in_maps, core_ids=list(range(N_CORES)), **kw)
    out = np.zeros((1, 4, N_T, N_IN), np.float32)
    for b in range(4):
        p0 = np.asarray(res.results[2 * b]["out"]).astype(np.float32)
        p1 = np.asarray(res.results[2 * b + 1]["out"]).astype(np.float32)
        out[0, b] = (p0 + p1).reshape(N_T, N_IN)
    return out, res


def kernel(r_prime, Q, E):
    out, _ = _run(r_prime, Q, E, trace=False)
    return out
